# revision 13
# baseline (speedup 1.0000x reference)
"""APPNP GNN forward on 8 Trainium2 NeuronCores (Bass/Tile, SPMD).

Strategy (all 8 cores share one instruction stream; per-core data differs):
  - nodes sharded 12500/core; MLP data-parallel; z fp16 [128, NCHK*64] SBUF
  - z mirrored to HBM as FOUR quarter-tables (24/24/24/26 chunks), 256B
    rows, double-buffered by step parity; each quarter is AllGather'd as
    soon as its chunks are computed, so AGs overlap compute and the next
    step's gathers on earlier quarters
  - a quarter-table spans all 8 cores within int16 range, so gathers index
    the whole table (no block windows); region == quarter
  - edges partitioned by dst core; per (dst chunk, quarter) runs unaligned
    (SPMD max only); tiles of 128 slots span chunk boundaries
  - slots gathered via dma_gather (int16 row ids, 256B rows), queue=quarter
  - S matrices streamed from HBM as fp8e4m3 per-SEGMENT [128,128] tiles
    (one segment per chunk x tile overlap; rows outside the segment have
    w=0 so every matmul is a full-128 masked one)
  - aggregation via TensorE into per-chunk PSUM [128, 64]; alpha*h and the
    self-loop term are applied by DVE at copy-out (no init matmuls)
  - final iteration computes log_softmax on-chip from PSUM, fp32 out
"""
import sys, os, types

sys.path.insert(0, "/opt/trn_rl_repo")
import numpy as np

N = 100000
NCORE = 8
NPC = N // NCORE             # 12500
CH = 128
NCHK = (NPC + CH - 1) // CH  # 98
QB = (0, 24, 48, 72, 98)     # quarter chunk boundaries
NRQ = (24, 24, 24, 26)       # chunks per quarter
GSIZE = 8
ALPHA = 0.1
MAX_CALL_TILES = 40
K_ITERS = 10
F_IN = 128
HID = 256
CLS = 47
NREG = 4                     # regions == quarters
AG_AFTER_GROUP = (2, 5, 8, 12)
SINGLE_PACKET = False
S_FP8 = True

TRACE = False           # set by test harness for NTFF profiling
LAST_EXEC_NS = None
LAST_SCOPES = None


def _chunk_size(i):
    return min(CH, NPC - CH * i)


def _preprocess(edge_index):
    import ml_dtypes

    src = np.asarray(edge_index[0], dtype=np.int64)
    dst = np.asarray(edge_index[1], dtype=np.int64)

    deg = np.bincount(dst, minlength=N).astype(np.float64) + 1.0
    dinv = 1.0 / np.sqrt(deg)
    ew = (dinv[src] * dinv[dst]) * (1.0 - ALPHA)
    selfw = ((dinv * dinv) * (1.0 - ALPHA)).astype(np.float32)

    core_d = dst // NPC
    dloc = dst % NPC
    chunk = dloc >> 7
    dcol = dloc & 127
    core_s = src // NPC
    sl = src % NPC
    p = sl & 127
    isrc = sl >> 7
    qb = np.asarray(QB)
    region = np.searchsorted(qb, isrc, side="right") - 1
    nrq = np.asarray(NRQ)[region]
    rowloc = core_s * (128 * nrq) + p * nrq + (isrc - qb[region])

    order = np.lexsort((rowloc, region, chunk, core_d))
    core_so = core_d[order]
    chunk_o = chunk[order]; region_o = region[order]
    dcol_o = dcol[order]; rowloc_o = rowloc[order]
    w_o = ew[order].astype(np.float32)

    key = (core_so * NCHK + chunk_o) * NREG + region_o
    nkeys = NCORE * NCHK * NREG
    starts = np.searchsorted(key, np.arange(nkeys), side="left")
    ends = np.searchsorted(key, np.arange(nkeys), side="right")
    cnts = (ends - starts).reshape(NCORE, NCHK, NREG)
    alloc_ir = cnts.max(axis=0)                  # [NCHK, NREG], unaligned

    groups = []
    i = 0
    while i < NCHK:
        groups.append(list(range(i, min(i + GSIZE, NCHK))))
        i += GSIZE

    # --- slot layout: per (group, region) concat runs, pad to 128 --------
    tile_call = []
    tile_tloc = []
    calls = []
    run_slot = {}
    raw_segs = []             # (tile, lo, hi, chunk) in (tile, lo) order
    calls_of_group = {}
    nslot = 0

    for gi, g in enumerate(groups):
        cg = []
        for r in range(NREG):
            gr_t0 = len(tile_call)
            base = nslot
            off = 0
            spans = []
            for i_ in g:
                run_slot[(i_, r)] = base + off
                spans.append((i_, off, off + alloc_ir[i_, r]))
                off += alloc_ir[i_, r]
            tot = ((off + 127) // 128) * 128
            ntile = tot // 128
            nslot = base + tot
            ncall = max(1, -(-ntile // MAX_CALL_TILES))
            sizes = [ntile // ncall + (1 if k < ntile % ncall else 0)
                     for k in range(ncall)]
            t = gr_t0
            for snt in sizes:
                cid = len(calls)
                calls.append({"t0": t, "nt": snt, "region": r, "group": gi})
                for k in range(snt):
                    tile_call.append(cid)
                    tile_tloc.append(k)
                t += snt
                cg.append(cid)
            for (i_, a, b) in spans:
                if a == b:
                    continue
                ta = a // 128
                while 128 * ta < b:
                    lo = max(a, 128 * ta) - 128 * ta
                    hi = min(b, 128 * (ta + 1)) - 128 * ta
                    raw_segs.append((gr_t0 + ta, lo, hi, i_))
                    ta += 1
        calls_of_group[gi] = cg

    NT = len(tile_call)
    SLOTS = NT * CH
    assert nslot == SLOTS

    # segments: global order by (tile, lo); contiguous per call
    raw_segs.sort(key=lambda x: (x[0], x[1]))
    seg_list = [(t, lo, hi) for (t, lo, hi, _) in raw_segs]
    seg_by_chunk = {}
    for sid, (t, lo, hi, i_) in enumerate(raw_segs):
        seg_by_chunk.setdefault(i_, []).append(sid)
    NSEG = len(seg_list)
    seg_lo = {}
    seg_cnt = {}
    for sid, (t, lo, hi) in enumerate(seg_list):
        cid = tile_call[t]
        seg_lo.setdefault(cid, sid)
        seg_cnt[cid] = seg_cnt.get(cid, 0) + 1
    for cid, call in enumerate(calls):
        call["s0"] = seg_lo[cid]
        call["ns"] = seg_cnt[cid]
    MAXSEGC = max(c["ns"] for c in calls)

    # --- per-core slot fills -------------------------------------------
    idx16 = np.zeros((NCORE, SLOTS), np.int16)
    dcol_f = np.zeros((NCORE, SLOTS), np.int64)
    w_f = np.zeros((NCORE, SLOTS), np.float32)
    for c in range(NCORE):
        for i_ in range(NCHK):
            for r in range(NREG):
                k = (c * NCHK + i_) * NREG + r
                s0, s1 = starts[k], ends[k]
                n = s1 - s0
                if n == 0:
                    continue
                b0 = run_slot[(i_, r)]
                idx16[c, b0:b0 + n] = rowloc_o[s0:s1].astype(np.int16)
                dcol_f[c, b0:b0 + n] = dcol_o[s0:s1]
                w_f[c, b0:b0 + n] = w_o[s0:s1]

    # idx wrapped into 16 partitions, replicated 8x across 128; per call
    idx_sb = np.zeros((NCORE, CH, SLOTS // 16), np.int16)
    off16 = 0
    for call in calls:
        call["idx_off16"] = off16
        nsl = call["nt"] * CH
        s0 = call["t0"] * CH
        for c in range(NCORE):
            seg = idx16[c, s0:s0 + nsl]
            idx_sb[c, :, off16:off16 + nsl // 16] = np.tile(
                seg.reshape(nsl // 16, 16).T, (8, 1))
        off16 += nsl // 16

    # --- dense per-segment S tiles, fp8e4m3: [core, 128, NSEG*128] ------
    sdt = ml_dtypes.float8_e4m3 if S_FP8 else np.float16
    sdata = np.zeros((NCORE, CH, NSEG, CH), sdt)
    wq = w_f.astype(sdt)
    for sid, (t, lo, hi) in enumerate(seg_list):
        s0 = t * CH
        rows = np.arange(lo, hi)
        for c in range(NCORE):
            sdata[c, rows, sid, dcol_f[c, s0 + lo:s0 + hi]] = \
                wq[c, s0 + lo:s0 + hi]
    sdata = np.ascontiguousarray(sdata.reshape(NCORE, CH, NSEG * CH))

    selfw_sb = np.zeros((NCORE, CH, NCHK), np.float32)
    for c in range(NCORE):
        sw = selfw[c * NPC:(c + 1) * NPC]
        swp = np.zeros(NCHK * CH, np.float32)
        swp[:NPC] = sw
        selfw_sb[c] = swp.reshape(NCHK, CH).T

    static = {"groups": groups, "calls": calls,
              "calls_of_group": calls_of_group,
              "seg_by_chunk": seg_by_chunk, "seg_list": seg_list,
              "tile_call": tile_call, "tile_tloc": tile_tloc,
              "NT": NT, "SLOTS": SLOTS, "NSEG": NSEG, "MAXSEGC": MAXSEGC}
    per_core = {"idx_sb": idx_sb, "sdata": sdata, "selfw_sb": selfw_sb}
    return static, per_core


def _install_ntff_hook():
    from concourse import bass_utils
    try:
        import antenv
        from trn_agent_boot.trn_boot import _ntff_profile_via_ctypes
    except Exception:
        return
    if "antenv.axon_hooks" in sys.modules:
        return
    mod = types.ModuleType("antenv.axon_hooks")
    state = {"hook": None}
    mod.set_axon_ntff_profile_hook = lambda h: state.__setitem__("hook", h)
    mod.get_axon_ntff_profile_hook = lambda: state["hook"]
    sys.modules["antenv.axon_hooks"] = mod
    antenv.axon_hooks = mod
    mod.set_axon_ntff_profile_hook(
        _ntff_profile_via_ctypes("/opt/axon/libaxon_pjrt.so"))
    bass_utils.upload_artifacts = lambda tmpdir: f"local:{tmpdir}"


def _build(static):
    import concourse.bass as bass
    import concourse.bacc as bacc
    import concourse.tile as tile
    import concourse.mybir as mybir
    from concourse.masks import make_identity

    f32 = mybir.dt.float32
    f16 = mybir.dt.float16
    f8 = mybir.dt.float8e4 if S_FP8 else mybir.dt.float16
    i16 = mybir.dt.int16
    AF = mybir.ActivationFunctionType
    OP = mybir.AluOpType
    AX = mybir.AxisListType

    groups = static["groups"]
    calls = static["calls"]
    calls_of_group = static["calls_of_group"]
    seg_by_chunk = static["seg_by_chunk"]
    seg_list = static["seg_list"]
    tile_call = static["tile_call"]
    tile_tloc = static["tile_tloc"]
    SLOTS = static["SLOTS"]
    NSEG = static["NSEG"]
    MAXSEGC = static["MAXSEGC"]
    seg_chunk = {}
    for i_, sl_ in seg_by_chunk.items():
        for sid in sl_:
            seg_chunk[sid] = i_

    nc = bacc.Bacc("TRN2", target_bir_lowering=False, debug=False,
                   num_devices=NCORE, num_swdge_queues=4)

    x_d = nc.dram_tensor("x_sh", [NPC, F_IN], f32, kind="ExternalInput").ap()
    W1_d = nc.dram_tensor("w1", [F_IN, HID], f32, kind="ExternalInput").ap()
    W2_d = nc.dram_tensor("w2", [HID, CLS], f32, kind="ExternalInput").ap()
    b1_d = nc.dram_tensor("b1c", [128, 2], f32, kind="ExternalInput").ap()
    b2_d = nc.dram_tensor("b2r", [128, CLS], f32, kind="ExternalInput").ap()
    idx_d = nc.dram_tensor("idxs", [128, SLOTS // 16], i16,
                           kind="ExternalInput").ap()
    sdata_d = nc.dram_tensor("sdata", [128, NSEG * 128], f8,
                             kind="ExternalInput").ap()
    selfw_d = nc.dram_tensor("selfwf", [128, NCHK], f32,
                             kind="ExternalInput").ap()
    out_d = nc.dram_tensor("out", [NPC, CLS], f32, kind="ExternalOutput").ap()

    hstage = [[nc.dram_tensor(f"hstage{q}_{pa}", [128, NRQ[q] * 128], f16).ap()
               for q in range(4)] for pa in range(2)]
    ztab = [[nc.dram_tensor(f"ztab{q}_{pa}", [NCORE * 128 * NRQ[q], 128],
                            f16, addr_space="Shared").ap()
             for q in range(4)] for pa in range(2)]

    with tile.TileContext(nc) as tc:
        with (
            tc.tile_pool(name="const", bufs=1) as cp,
            tc.tile_pool(name="resident", bufs=1) as rp,
            tc.tile_pool(name="mlp", bufs=3) as mp,
            tc.tile_pool(name="gb", bufs=6) as gp,
            tc.tile_pool(name="sb", bufs=6) as sp,
            tc.tile_pool(name="sm", bufs=4) as smp,
        ):
            # constants / residents
            idx_t = rp.tile([128, SLOTS // 16], i16)
            nc.sync.dma_start(idx_t[:], idx_d[:])
            selfw_t = rp.tile([128, NCHK], f32)
            nc.sync.dma_start(selfw_t[:], selfw_d[:])
            W1_t = cp.tile([128, HID], f32)
            nc.sync.dma_start(W1_t[:], W1_d[:])
            W2a_t = cp.tile([128, CLS], f32)
            nc.sync.dma_start(W2a_t[:], W2_d[0:128, :])
            W2b_t = cp.tile([128, CLS], f32)
            nc.sync.dma_start(W2b_t[:], W2_d[128:256, :])
            b1_t = cp.tile([128, 2], f32)
            nc.sync.dma_start(b1_t[:], b1_d[:])
            b2_t = cp.tile([128, CLS], f32)
            nc.sync.dma_start(b2_t[:], b2_d[:])
            ident = cp.tile([128, 128], f32)
            make_identity(nc, ident[:])

            ah_t = rp.tile([128, NCHK * 64], f16)
            nc.vector.memset(ah_t[:], 0.0)
            stgA = rp.tile([128, NCHK * 64], f16)
            nc.vector.memset(stgA[:], 0.0)
            stgB = rp.tile([128, NCHK * 64], f16)
            nc.vector.memset(stgB[:], 0.0)

            # ---- MLP: z0 = relu(x@W1+b1)@W2+b2 ----
            with tc.tile_pool(name="psmlp", bufs=2, space="PSUM") as pmp:
                for i in range(NCHK):
                    sz = _chunk_size(i)
                    xt = mp.tile([128, F_IN], f32, tag="xt")
                    nc.sync.dma_start(xt[0:sz, :], x_d[CH * i:CH * i + sz, :])
                    pxT = pmp.tile([128, 128], f32, tag="pmlp")
                    nc.tensor.transpose(pxT[:, 0:sz], xt[0:sz, :],
                                        ident[0:sz, 0:sz])
                    xT = mp.tile([128, 128], f32, tag="xT")
                    nc.scalar.activation(xT[:, 0:sz], pxT[:, 0:sz], AF.Copy)
                    relus = []
                    for h in range(2):
                        ph = pmp.tile([128, 128], f32, tag="pmlp")
                        nc.tensor.matmul(ph[:, 0:sz],
                                         lhsT=W1_t[:, 128 * h:128 * (h + 1)],
                                         rhs=xT[:, 0:sz], start=True,
                                         stop=True)
                        rh = mp.tile([128, 128], f32, tag=f"relu{h}")
                        nc.scalar.activation(rh[:, 0:sz], ph[:, 0:sz],
                                             AF.Relu, bias=b1_t[:, h:h + 1])
                        relus.append(rh)
                    pz = pmp.tile([128, 128], f32, tag="pmlp")
                    for h in range(2):
                        nc.tensor.matmul(pz[0:sz, 0:CLS],
                                         lhsT=relus[h][:, 0:sz],
                                         rhs=(W2a_t if h == 0 else W2b_t)[:],
                                         start=(h == 0), stop=(h == 1))
                    z0 = mp.tile([128, CLS], f32, tag="z0")
                    nc.vector.tensor_tensor(out=z0[0:sz, :],
                                            in0=pz[0:sz, 0:CLS],
                                            in1=b2_t[0:sz, :], op=OP.add)
                    nc.vector.tensor_copy(
                        out=stgA[0:sz, 64 * i:64 * i + CLS], in_=z0[0:sz, :])
                    nc.scalar.mul(ah_t[0:sz, 64 * i:64 * i + CLS],
                                  z0[0:sz, :], ALPHA)

            def stage_group(stg, pa, gi):
                g = groups[gi]
                q = 0
                while g[0] >= QB[q + 1]:
                    q += 1
                i0 = g[0] - QB[q]
                i1 = g[-1] + 1 - QB[q]
                hv = hstage[pa][q][:].rearrange("p (i f) -> p i f", f=128)
                sv = stg[:].rearrange("p (i f) -> p i f", f=64)
                nc.sync.dma_start(hv[:, i0:i1, 0:64],
                                  sv[:, g[0]:g[-1] + 1, :])

            def do_ag(pa, q):
                nc.gpsimd.collective_compute(
                    "AllGather", mybir.AluOpType.bypass,
                    replica_groups=[list(range(NCORE))],
                    ins=[hstage[pa][q][:].opt()],
                    outs=[ztab[pa][q][:].opt()])

            for gi in range(len(groups)):
                stage_group(stgA, 1, gi)
                if gi in AG_AFTER_GROUP:
                    do_ag(1, AG_AFTER_GROUP.index(gi))

            # ---- K propagation steps ----
            stg_prev, stg_new = stgA, stgB
            with tc.tile_pool(name="pschunk", bufs=8, space="PSUM") as psp:
                for k in range(1, K_ITERS + 1):
                    pa = k % 2
                    npa = (k + 1) % 2
                    for gi, grp in enumerate(groups):
                        gtile = {}
                        stile = {}
                        for cid in calls_of_group[gi]:
                            call = calls[cid]
                            nt = call["nt"]
                            r = call["region"]
                            g = gp.tile([128, MAX_CALL_TILES, 128], f16,
                                        tag="g")
                            nc.gpsimd.dma_gather(
                                g[:, 0:nt, :],
                                ztab[pa][r][:],
                                idx_t[:, call["idx_off16"]:
                                      call["idx_off16"] + nt * 8],
                                nt * 128, nt * 128, 128,
                                single_packet=SINGLE_PACKET,
                                queue_num=r,
                            )
                            gtile[cid] = g
                            st = sp.tile([128, MAXSEGC * 128], f8, tag="st")
                            ns = call["ns"]
                            s0 = call["s0"]
                            nc.sync.dma_start(
                                st[:, 0:ns * 128],
                                sdata_d[:, s0 * 128:(s0 + ns) * 128])
                            stile[cid] = st
                        # per-chunk PSUM tiles; matmuls issued TILE-major
                        # (arrival order) with interleaved accum groups so
                        # the PE consumes gathers as they land
                        pstile = {}
                        first = {}
                        last = {}
                        for i in grp:
                            pst = psp.tile([128, 64], f32, tag="ps")
                            pstile[i] = pst
                            segs = seg_by_chunk[i]
                            first[i] = segs[0]
                            last[i] = segs[-1]
                        for cid in calls_of_group[gi]:
                            call = calls[cid]
                            for sj in range(call["ns"]):
                                sid = call["s0"] + sj
                                (t, lo, hi) = seg_list[sid]
                                i = seg_chunk[sid]
                                tloc = tile_tloc[t]
                                nc.tensor.matmul(
                                    pstile[i][:, :],
                                    lhsT=stile[cid][:, 128 * sj:
                                                    128 * (sj + 1)],
                                    rhs=gtile[cid][:, tloc, 0:64],
                                    start=(sid == first[i]),
                                    stop=(sid == last[i]),
                                    skip_group_check=True)
                        for i in grp:
                            sz = _chunk_size(i)
                            ps = pstile[i]
                            # combine: z = ps + selfw*(1-a)*z_prev + a*h
                            t1 = smp.tile([128, 64], f16, tag="t1")
                            nc.vector.tensor_scalar(
                                out=t1[0:sz, :],
                                in0=stg_prev[0:sz, 64 * i:64 * (i + 1)],
                                scalar1=selfw_t[0:sz, i:i + 1],
                                scalar2=None, op0=OP.mult)
                            t2 = smp.tile([128, 64], f16, tag="t2")
                            nc.vector.tensor_tensor(
                                out=t2[0:sz, :], in0=t1[0:sz, :],
                                in1=ah_t[0:sz, 64 * i:64 * (i + 1)],
                                op=OP.add)
                            if k < K_ITERS:
                                nc.vector.tensor_tensor(
                                    out=stg_new[0:sz, 64 * i:64 * (i + 1)],
                                    in0=ps[0:sz, 0:64], in1=t2[0:sz, :],
                                    op=OP.add)
                            else:
                                z = smp.tile([128, CLS], f32, tag="z")
                                nc.vector.tensor_tensor(
                                    out=z[0:sz, :], in0=ps[0:sz, 0:CLS],
                                    in1=t2[0:sz, 0:CLS], op=OP.add)
                                m = smp.tile([128, 1], f32, tag="m")
                                nc.vector.tensor_reduce(
                                    m[0:sz, :], z[0:sz, :], axis=AX.X,
                                    op=OP.max)
                                nm = smp.tile([128, 1], f32, tag="nm")
                                nc.vector.tensor_scalar_mul(
                                    nm[0:sz, :], m[0:sz, :], -1.0)
                                e = smp.tile([128, CLS], f32, tag="e")
                                nc.scalar.activation(e[0:sz, :], z[0:sz, :],
                                                     AF.Exp,
                                                     bias=nm[0:sz, 0:1])
                                s = smp.tile([128, 1], f32, tag="s")
                                nc.vector.tensor_reduce(
                                    s[0:sz, :], e[0:sz, :], axis=AX.X,
                                    op=OP.add)
                                ls = smp.tile([128, 1], f32, tag="ls")
                                nc.scalar.activation(ls[0:sz, :], s[0:sz, :],
                                                     AF.Ln)
                                offs = smp.tile([128, 1], f32, tag="offs")
                                nc.vector.tensor_tensor(
                                    out=offs[0:sz, :], in0=m[0:sz, :],
                                    in1=ls[0:sz, :], op=OP.add)
                                res = smp.tile([128, CLS], f32, tag="res")
                                nc.vector.tensor_scalar(
                                    out=res[0:sz, :], in0=z[0:sz, :],
                                    scalar1=offs[0:sz, 0:1], scalar2=None,
                                    op0=OP.subtract)
                                nc.sync.dma_start(
                                    out_d[CH * i:CH * i + sz, :],
                                    res[0:sz, :])
                        if k < K_ITERS:
                            stage_group(stg_new, npa, gi)
                            if gi in AG_AFTER_GROUP:
                                do_ag(npa, AG_AFTER_GROUP.index(gi))
                    if k < K_ITERS:
                        stg_prev, stg_new = stg_new, stg_prev

    nc.compile()
    return nc


def kernel(x, edge_index, W1, b1, W2, b2):
    global LAST_EXEC_NS, LAST_SCOPES
    from concourse import bass_utils

    x = np.asarray(x, np.float32)
    ei = np.asarray(edge_index)
    W1 = np.asarray(W1, np.float32)
    b1 = np.asarray(b1, np.float32)
    W2 = np.asarray(W2, np.float32)
    b2 = np.asarray(b2, np.float32)

    static, per_core = _preprocess(ei)
    nc = _build(static)

    b1c = np.stack([b1[0:128], b1[128:256]], axis=1).astype(np.float32)
    b1c = np.ascontiguousarray(b1c)
    b2r = np.ascontiguousarray(np.tile(b2[None, :], (128, 1)).astype(np.float32))

    in_maps = []
    for c in range(NCORE):
        in_maps.append({
            "x_sh": np.ascontiguousarray(x[c * NPC:(c + 1) * NPC]),
            "w1": W1, "w2": W2, "b1c": b1c, "b2r": b2r,
            "idxs": np.ascontiguousarray(per_core["idx_sb"][c]),
            "sdata": np.ascontiguousarray(per_core["sdata"][c]),
            "selfwf": np.ascontiguousarray(per_core["selfw_sb"][c]),
        })

    if TRACE:
        _install_ntff_hook()
    res = bass_utils.run_bass_kernel_spmd(
        nc, in_maps, core_ids=list(range(NCORE)), trace=TRACE)
    LAST_EXEC_NS = res.exec_time_ns
    LAST_SCOPES = res.per_core_scope_times

    out = np.concatenate([res.results[c]["out"] for c in range(NCORE)], axis=0)
    return out.astype(np.float32)
